# revision 65
# baseline (speedup 1.0000x reference)
"""Fused self-attention + residual + LayerNorm kernel for Trainium2.

Reference computation (per batch b of 16):
    S    = x @ x.T                  [2048, 2048]
    A    = softmax(S, axis=-1)
    out  = A @ x                    [2048, 128]
    y    = out + x
    res  = LayerNorm(y) * gamma + beta
Sharding: data-parallel over batch, 2 batches per core on 8 NeuronCores (SPMD,
no collectives).

Algorithm notes (per core / per batch):
  * Stabilized softmax without a max pass: P[q,k] = exp(S[q,k] - (c_q+G)/2)
    with c_q = ||x_q||^2 and G a mid-range constant (soft-max/soft-min of c
    via exp(+-c/6) sums + float-bit ln).  Cauchy-Schwarz bounds the exponent
    by (c_k - G)/2, safely inside bf16/f32 range for G mid-range.
  * No explicit transpose or symmetrization anywhere: the AV matmul uses the
    q-major P tiles directly as lhsT.  Since S is symmetric, tile (j,jj) of P
    equals exp(S[q',k] - (c_k+G)/2) for q' in block jj, k in block j -- so
    with scaled values Vt[k] = t_k x[k] (t_k = exp((c_k-G)/2)) the per-k
    factors cancel: the accumulated result is num'[q'] = t_q' * num[q'].
    The stray t_q' folds into the existing per-row normalization scalar
    R = 1/(den_q * t_q) -- the elementwise P->E multiply of the symmetric
    formulation disappears entirely.
  * exp runs on ACT (bias per-partition, accum_out = row-sum denominators
    for free) for most j-steps; a subset of steps instead computes P on the
    gpsimd engine with a two-op Schraudolph bit-trick
        w = max(S + bias_q, -88);  P_bits(int16) = trunc(A16*w + B16)
    bitcast to bf16 (A16 = 128/ln2).  The clamp keeps the int16 conversion
    out of the NaN band; softmax normalization cancels the ~3% per-entry
    error.  Denominators for those steps come from a DVE row-reduce of P.
    This splits the exp workload across ACT/Pool/DVE so the tensor engine
    (QK^T + AV at their streaming rooflines) becomes the bottleneck.
  * QK^T and AV run in bf16 (f32 PSUM accumulation); rsqrt for LayerNorm is
    fast-inverse-sqrt + 2 Newton steps so ACT stays on one table set.
  * The two batches are software-pipelined: batch 1's main loop overlaps
    batch 0's output stage, and each engine's issue order is time-monotone.
"""

import sys

import numpy as np

sys.path.insert(0, "/opt/trn_rl_repo")

B, T, D = 16, 2048, 128
N_CORES = 8
NB = B // N_CORES          # batches per core
NT = T // 128              # 128-row tiles per batch
EPS = 1e-5

# per-slab engine for the 64 512-wide exp slabs (slab s = 4*j + h):
# 'A' = ACT LUT exp, 'P' = gpsimd two-op Schraudolph.  Pattern [A,P,A]
# repeating = 43 ACT + 21 Pool, so ACT and Pool consume adjacent S slabs
# concurrently instead of starving each other.  Denominators cost nothing
# here: they accumulate on the PE as N=1 matmuls (rhs = t column) reusing
# the AV matmuls' stationary weights, into a dedicated PSUM bank.
MODE = {s: ("P" if (s % 3 == 2 and s >= 2) else "A") for s in range(64)}
A16 = 128.0 / 0.6931471805599453        # bf16 Schraudolph scale
B16 = 16251.0                           # 127*128 - minimax shift + trunc comp
LN2 = 0.6931471805599453

_CACHE = {}


def _build():
    from contextlib import ExitStack

    import concourse.bacc as bacc
    import concourse.bass as bass  # noqa: F401
    import concourse.tile as tile
    from concourse import mybir

    f32 = mybir.dt.float32
    bf = mybir.dt.bfloat16
    i16 = mybir.dt.int16
    AF = mybir.ActivationFunctionType
    ALU = mybir.AluOpType
    AX = mybir.AxisListType

    nc = bacc.Bacc()

    x_d = nc.dram_tensor("x", [NB, T, D], bf, kind="ExternalInput")
    xT_d = nc.dram_tensor("xT", [NB, D, T], bf, kind="ExternalInput")
    g_d = nc.dram_tensor("gamma", [D], f32, kind="ExternalInput")
    b_d = nc.dram_tensor("beta", [D], f32, kind="ExternalInput")
    o_d = nc.dram_tensor("out", [NB, T, D], f32, kind="ExternalOutput")

    ctx = ExitStack()
    with tile.TileContext(nc) as tc, ctx:
        big = ctx.enter_context(tc.tile_pool(name="big", bufs=2))
        epool = ctx.enter_context(tc.tile_pool(name="epool", bufs=12))
        stats = ctx.enter_context(tc.tile_pool(name="stats", bufs=2))
        consts = ctx.enter_context(tc.tile_pool(name="consts", bufs=1))
        spool = ctx.enter_context(tc.tile_pool(name="spool", bufs=3, space="PSUM"))
        npool = ctx.enter_context(tc.tile_pool(name="npool", bufs=1, space="PSUM"))
        dpool = ctx.enter_context(tc.tile_pool(name="dpool", bufs=1, space="PSUM"))

        zero_t = consts.tile([128, 1], f32, tag="zero", name="zero")
        nc.vector.memset(zero_t, 0.0)
        ones_c = consts.tile([128, 1], f32, tag="ones_c", name="ones_c")
        nc.vector.memset(ones_c, 1.0)
        ones_r = consts.tile([1, 128], f32, tag="ones_r", name="ones_r")
        nc.vector.memset(ones_r, 1.0)

        def emit_loads(b, st):
            st["xT"] = big.tile([128, T], bf, tag="xT", name="xT")
            st["x"] = big.tile([128, NT, D], bf, tag="x", name="x")
            xv = x_d[b].rearrange("(t p) d -> p t d", p=128)
            for sx in range(4):
                nc.sync.dma_start(
                    out=st["x"][:, sx * 4 : (sx + 1) * 4, :],
                    in_=xv[:, sx * 4 : (sx + 1) * 4, :],
                )

        def emit_loads_xT(b, st):
            # b0: halves on the ACT and Pool DGE queues so the Pool sqb
            # chain only waits for one half
            engs = (nc.scalar, nc.gpsimd) if b == 0 else (nc.sync, nc.sync)
            for sx in range(2):
                engs[sx].dma_start(
                    out=st["xT"][:, sx * 1024 : (sx + 1) * 1024],
                    in_=xT_d[b, :, sx * 1024 : (sx + 1) * 1024],
                )

        def emit_stats_a(b, st):
            # x row-norms: gpsimd squares; the per-slice DVE reduces are in
            # emit_stats_b's quarter loop so wait-hints keep them off the
            # G-chain's critical path
            x_sb = st["x"]
            st["C"] = stats.tile([128, NT], f32, tag="C", name="C")
            st["bias"] = stats.tile([128, NT], f32, tag="bias", name="bias")
            st["sqb"] = sqb = big.tile([128, NT, D], f32, tag="sqb", name="sqb")
            for t in range(NT):
                nc.gpsimd.tensor_mul(
                    out=sqb[:, t, :], in0=x_sb[:, t, :], in1=x_sb[:, t, :]
                )
            nc.vector.tensor_reduce(
                out=st["C"][:, 0:4], in_=sqb[:, 0:4, :], axis=AX.X, op=ALU.add
            )

        def emit_stats_b(b, st):
            # Soft bounds on the range of c without cross-partition reductions:
            #   cbar ~ 6 ln(sum exp(c/6)),  mbar ~ -6 ln(sum exp(-c/6))
            # over the FIRST 4 row-tiles only (512 rows bound the range within
            # a few units -- the +-75 exponent margins absorb that), so the
            # G chain starts after the first x DMA slice.  The ln's use
            # Schraudolph float-bits; cross-partition sum and the broadcast
            # back are K=1/M=1,2 matmuls on the PE into the den PSUM bank.
            C = st["C"]
            ec2 = stats.tile([128, 2], f32, tag="ec2", name="ec2")
            EC = stats.tile([128, 4], f32, tag="EC", name="EC")
            nc.scalar.activation(
                out=EC, in_=C[:, 0:4], func=AF.Exp, bias=zero_t, scale=1.0 / 6.0,
                accum_out=ec2[:, 0:1],
            )
            ECm = stats.tile([128, 4], f32, tag="ECm", name="ECm")
            nc.scalar.activation(
                out=ECm, in_=C[:, 0:4], func=AF.Exp, bias=zero_t, scale=-1.0 / 6.0,
                accum_out=ec2[:, 1:2],
            )
            s1a = spool.tile([1, 1], f32, tag="S", name="s1a")
            nc.tensor.matmul(out=s1a, lhsT=ec2[:, 0:1], rhs=ones_c, start=True, stop=True)
            s1b = spool.tile([1, 1], f32, tag="S", name="s1b")
            nc.tensor.matmul(out=s1b, lhsT=ec2[:, 1:2], rhs=ones_c, start=True, stop=True)
            LL2 = stats.tile([1, 2], f32, tag="LL2", name="LL2")
            nc.vector.tensor_copy(out=LL2[0:1, 0:1], in_=s1a.bitcast(mybir.dt.int32))
            nc.vector.tensor_copy(out=LL2[0:1, 1:2], in_=s1b.bitcast(mybir.dt.int32))
            s2 = spool.tile([128, 2], f32, tag="S", name="s2")
            nc.tensor.matmul(out=s2, lhsT=ones_r, rhs=LL2, start=True, stop=True)
            # -G/2 = (3*LN2/2^24)*(bits_minus - bits_plus)  [128,1]
            s2s = stats.tile([128, 2], f32, tag="s2s", name="s2s")
            nc.vector.tensor_copy(out=s2s, in_=s2)
            Gd = stats.tile([128, 1], f32, tag="Gd", name="Gd")
            nc.vector.tensor_tensor(
                out=Gd, in0=s2s[:, 1:2], in1=s2s[:, 0:1], op=ALU.subtract
            )
            Gh_neg = stats.tile([128, 1], f32, tag="Ghn", name="Ghn")
            nc.vector.tensor_scalar_mul(out=Gh_neg, in0=Gd, scalar1=1.5 * LN2 / 8388608.0)
            # per-quarter: remaining C reduces (wait-hinted off the critical
            # path), bias chunk, t_k = exp((c_k-G)/2), Vt = t*x, Tb = bf16 t.
            # The first quarter gates the first exps/AVs; later quarters only
            # need to beat their own j-steps.
            st["Ghn"] = Gh_neg
            st["Tall"] = stats.tile([128, NT], f32, tag="Tall", name="Tall")
            st["Vt"] = big.tile([128, NT, D], bf, tag="Vt", name="Vt")
            st["Tb"] = stats.tile([128, NT], bf, tag="Tb", name="Tb")
            emit_stats_q(b, st, 0)

        def emit_stats_q(b, st, qd):
            # quarter qd of: C reduce, bias chunk, t = exp((c-G)/2), bf16 t,
            # Vt = t*x.  Emitted right before j-step 4*qd uses them, so their
            # emission priority can't preempt the head's critical chain.
            C, Gh_neg, Tall = st["C"], st["Ghn"], st["Tall"]
            cs = slice(4 * qd, 4 * qd + 4)
            if qd > 0:
                nc.vector.tensor_reduce(
                    out=C[:, cs], in_=st["sqb"][:, cs, :], axis=AX.X, op=ALU.add
                )
            nc.vector.tensor_scalar(
                out=st["bias"][:, cs],
                in0=C[:, cs],
                scalar1=-0.5,
                scalar2=Gh_neg,
                op0=ALU.mult,
                op1=ALU.add,
            )
            nc.scalar.activation(
                out=Tall[:, cs], in_=C[:, cs], func=AF.Exp, bias=Gh_neg, scale=0.5
            )
            nc.vector.tensor_copy(out=st["Tb"][:, cs], in_=Tall[:, cs])
            for t in range(4 * qd, 4 * qd + 4):
                nc.gpsimd.tensor_scalar_mul(
                    out=st["Vt"][:, t, :], in0=st["x"][:, t, :],
                    scalar1=Tall[:, t : t + 1],
                )

        def emit_qk_exp(b, st, j):
            # QK^T for row-block j + exp into four bf16 quarter-tiles.  The AV
            # matmuls for block j are emitted one step later (emit_av) so PE
            # work overlaps the exp of the next block instead of gating it.
            if j == 0:
                st["num"] = npool.tile([128, T], f32, tag="num", name="num")
                st["denP"] = dpool.tile([128, NT], f32, tag="denP", name="denP")
                st["E"] = {}
            Eq = [
                epool.tile([128, 512], bf, tag="E", name="E") for _ in range(4)
            ]
            st["E"][j] = Eq
            xT_sb = st["xT"]
            for h in range(4):
                S = spool.tile([128, 512], f32, tag="S", name="S")
                nc.tensor.matmul(
                    out=S,
                    lhsT=xT_sb[:, j * 128 : (j + 1) * 128],
                    rhs=xT_sb[:, h * 512 : (h + 1) * 512],
                    start=True,
                    stop=True,
                )
                if MODE[4 * j + h] == "P":
                    # DVE+Pool 2-op Schraudolph: w = max(S+bias,-88) on DVE
                    # (gpsimd cannot read PSUM); bits = trunc(A16*w + B16) on
                    # gpsimd -> int16 view of bf16 tile
                    tmp = big.tile([128, 512], f32, tag="stmp", name="stmp", bufs=3)
                    with tc.high_priority():
                        nc.vector.tensor_scalar(
                            out=tmp,
                            in0=S,
                            scalar1=st["bias"][:, j : j + 1],
                            scalar2=-88.0,
                            op0=ALU.add,
                            op1=ALU.max,
                        )
                    nc.gpsimd.tensor_scalar(
                        out=Eq[h].bitcast(i16),
                        in0=tmp,
                        scalar1=A16,
                        scalar2=B16,
                        op0=ALU.mult,
                        op1=ALU.add,
                    )
                else:
                    nc.scalar.activation(
                        out=Eq[h],
                        in_=S,
                        func=AF.Exp,
                        bias=st["bias"][:, j : j + 1],
                        scale=1.0,
                    )

        def emit_av(b, st, j):
            Eq = st["E"].pop(j)
            for jj in range(NT):
                lhsT = Eq[jj // 4][:, (jj % 4) * 128 : (jj % 4 + 1) * 128]
                # 4 output slices share a 2KB PSUM bank = one zero region:
                # only the bank's first MM sets start, only its last sets stop
                nc.tensor.matmul(
                    out=st["num"][:, jj * 128 : (jj + 1) * 128],
                    lhsT=lhsT,
                    rhs=st["Vt"][:, j, :],
                    start=(j == 0 and jj % 4 == 0),
                    stop=(j == NT - 1 and jj % 4 == 3),
                )
                # denominator: same stationary weights, t column as rhs ->
                # denP[q', jj] accumulates sum_k exp(S[q',k]-G) over all j
                nc.tensor.matmul(
                    out=st["denP"][:, jj : jj + 1],
                    lhsT=lhsT,
                    rhs=st["Tb"][:, j : j + 1],
                    start=(j == 0 and jj == 0),
                    stop=(j == NT - 1 and jj == NT - 1),
                )

        def emit_den(b, st):
            denS = stats.tile([128, NT], f32, tag="denS", name="denS")
            nc.vector.tensor_copy(out=denS, in_=st["denP"])
            R = stats.tile([128, NT], f32, tag="R", name="R")
            nc.vector.reciprocal(out=R, in_=denS)
            st["R"] = R

        def emit_drain(b, st, copy_psum=True, half=None):
            # drain AV results out of PSUM so the next batch can reuse it
            # (skipped for the last batch -- nothing needs the banks).
            # Staggered: half 0 at the phase boundary, half 1 a few iterations
            # later, so the copies don't starve DVE mid-phase.
            if copy_psum:
                if half != 1:
                    st["numS"] = big.tile([128, T], f32, tag="numS", name="numS")
                for h in ([0, 1] if half is None else [half]):
                    if h == 0:
                        nc.vector.tensor_copy(
                            out=st["numS"][:, 0:1024],
                            in_=st["num"][:, 0:1024],
                        )
                    else:
                        nc.scalar.activation(
                            out=st["numS"][:, 1024:2048],
                            in_=st["num"][:, 1024:2048],
                            func=AF.Copy,
                        )
            else:
                st["numS"] = st["num"]
            if half != 1:
                st["Y"] = big.tile([128, NT, D], f32, tag="Y", name="Y")
                st["Yout"] = big.tile([128, NT, D], f32, tag="Yout", name="Yout")
                st["Sy"] = stats.tile([128, NT], f32, tag="Sy", name="Sy")
                st["Sy2"] = stats.tile([128, NT], f32, tag="Sy2", name="Sy2")
                st["ysqb"] = big.tile([128, NT, D], f32, tag="ysqb", name="ysqb")

        def emit_outA(b, st, jj, tail=False):
            # y = num'/den' + x, with LN stats via accum_out.  The y-compute
            # reads numS (PSUM for the last batch) so it goes on DVE unless
            # numS was drained to SBUF, in which case odd tiles go on gpsimd.
            nc.vector.scalar_tensor_tensor(
                out=st["Y"][:, jj, :],
                in0=st["numS"][:, jj * 128 : (jj + 1) * 128],
                scalar=st["R"][:, jj : jj + 1],
                in1=st["x"][:, jj, :],
                op0=ALU.mult,
                op1=ALU.add,
                accum_out=st["Sy"][:, jj : jj + 1],
            )
            if tail and jj % 2 == 0:
                nc.scalar.activation(
                    out=st["ysqb"][:, jj, :],
                    in_=st["Y"][:, jj, :],
                    func=AF.Square,
                    accum_out=st["Sy2"][:, jj : jj + 1],
                )
            else:
                nc.vector.scalar_tensor_tensor(
                    out=st["ysqb"][:, jj, :],
                    in0=st["Y"][:, jj, :],
                    scalar=1.0,
                    in1=st["Y"][:, jj, :],
                    op0=ALU.mult,
                    op1=ALU.mult,
                    accum_out=st["Sy2"][:, jj : jj + 1],
                )

        def emit_lnr(b, st, lo=0, hi=NT):
            cs = slice(lo, hi)
            if True:
                if "mu" not in st:
                    st["mu"] = stats.tile([128, NT], f32, tag="mu", name="mu")
                    st["vart"] = stats.tile([128, NT], f32, tag="vart", name="vart")
                    st["rstd"] = stats.tile([128, NT], f32, tag="rstd", name="rstd")
                # mu = Sy/128, var = Sy2/128 - mu^2
                nc.vector.tensor_scalar_mul(
                    out=st["mu"][:, cs], in0=st["Sy"][:, cs], scalar1=1.0 / D
                )
                musq = stats.tile([128, NT], f32, tag="musq", name="musq")
                nc.vector.scalar_tensor_tensor(
                    out=musq[:, cs],
                    in0=st["mu"][:, cs],
                    scalar=1.0,
                    in1=st["mu"][:, cs],
                    op0=ALU.mult,
                    op1=ALU.mult,
                )
                nc.vector.scalar_tensor_tensor(
                    out=st["vart"][:, cs],
                    in0=st["Sy2"][:, cs],
                    scalar=1.0 / D,
                    in1=musq[:, cs],
                    op0=ALU.mult,
                    op1=ALU.subtract,
                )
                var_in = st["vart"][:, cs]
            # rstd = 1/sqrt(var+eps) via the fast-inverse-sqrt bit trick plus
            # two Newton steps (~4e-6 rel err) -- keeps the ACT engine on the
            # exp table set for the whole kernel (table swaps cost 1.3us each)
            ve = stats.tile([128, NT], f32, tag="ve", name="ve")
            nc.vector.tensor_scalar_add(out=ve[:, cs], in0=var_in, scalar1=EPS)
            wf = stats.tile([128, NT], f32, tag="wf", name="wf")
            nc.vector.tensor_copy(out=wf[:, cs], in_=ve[:, cs].bitcast(mybir.dt.int32))
            nc.vector.tensor_scalar(
                out=wf[:, cs], in0=wf[:, cs],
                scalar1=-0.5, scalar2=1597463007.0,
                op0=ALU.mult, op1=ALU.add,
            )
            wi = stats.tile([128, NT], mybir.dt.int32, tag="wi", name="wi")
            nc.vector.tensor_copy(out=wi[:, cs], in_=wf[:, cs])
            y = stats.tile([128, NT], f32, tag="y0", name="y0")
            nc.vector.tensor_copy(out=y[:, cs], in_=wi[:, cs].bitcast(f32))
            t1 = stats.tile([128, NT], f32, tag="t1", name="t1")
            for it in range(2):
                nc.vector.tensor_mul(out=t1[:, cs], in0=ve[:, cs], in1=y[:, cs])
                nc.vector.tensor_mul(out=t1[:, cs], in0=t1[:, cs], in1=y[:, cs])
                nc.vector.tensor_scalar(
                    out=t1[:, cs], in0=t1[:, cs],
                    scalar1=-0.5, scalar2=1.5, op0=ALU.mult, op1=ALU.add,
                )
                nc.vector.tensor_mul(
                    out=st["rstd"][:, cs] if it == 1 else y[:, cs],
                    in0=y[:, cs], in1=t1[:, cs],
                )

        def emit_outT(b, st, jj):
            # tail path: ACT scales num by R (idle engine there), Pool adds the
            # residual, DVE computes LN stats via bn_stats/bn_aggr
            nc.scalar.activation(
                out=st["Y1"][:, jj, :],
                in_=st["num"][:, jj * 128 : (jj + 1) * 128],
                func=AF.Copy,
                scale=st["R"][:, jj : jj + 1],
            )
            nc.gpsimd.tensor_add(
                out=st["Y"][:, jj, :], in0=st["Y1"][:, jj, :], in1=st["x"][:, jj, :]
            )
            bns = stats.tile([128, 6], f32, tag="bns", name="bns")
            nc.vector.bn_stats(out=bns, in_=st["Y"][:, jj, :])
            nc.vector.bn_aggr(out=st["MV"][:, jj, :], in_=bns)

        def emit_lnr_mv(b, st, lo=0, hi=NT):
            cs = slice(lo, hi)
            if "rstd" not in st:
                st["rstd"] = stats.tile([128, NT], f32, tag="rstd", name="rstd")
                st["ve"] = stats.tile([128, NT], f32, tag="ve", name="ve")
                st["wf"] = stats.tile([128, NT], f32, tag="wf", name="wf")
                st["wi"] = stats.tile([128, NT], mybir.dt.int32, tag="wi", name="wi")
                st["y0"] = stats.tile([128, NT], f32, tag="y0", name="y0")
                st["t1"] = stats.tile([128, NT], f32, tag="t1", name="t1")
            ve = st["ve"]
            nc.vector.tensor_scalar_add(out=ve[:, cs], in0=st["MV"][:, cs, 1], scalar1=EPS)
            wf = st["wf"]
            nc.vector.tensor_copy(out=wf[:, cs], in_=ve[:, cs].bitcast(mybir.dt.int32))
            nc.vector.tensor_scalar(
                out=wf[:, cs], in0=wf[:, cs],
                scalar1=-0.5, scalar2=1597463007.0,
                op0=ALU.mult, op1=ALU.add,
            )
            wi = st["wi"]
            nc.vector.tensor_copy(out=wi[:, cs], in_=wf[:, cs])
            y = st["y0"]
            nc.vector.tensor_copy(out=y[:, cs], in_=wi[:, cs].bitcast(f32))
            t1 = st["t1"]
            for it in range(2):
                nc.vector.tensor_mul(out=t1[:, cs], in0=ve[:, cs], in1=y[:, cs])
                nc.vector.tensor_mul(out=t1[:, cs], in0=t1[:, cs], in1=y[:, cs])
                nc.vector.tensor_scalar(
                    out=t1[:, cs], in0=t1[:, cs],
                    scalar1=-0.5, scalar2=1.5, op0=ALU.mult, op1=ALU.add,
                )
                nc.vector.tensor_mul(
                    out=st["rstd"][:, cs] if it == 1 else y[:, cs],
                    in0=y[:, cs], in1=t1[:, cs],
                )

        def emit_outB(b, st, jj, tail=False):
            mu_s = st["MV"][:, jj, 0:1] if "MV" in st else st["mu"][:, jj : jj + 1]
            z = stats.tile([128, D], f32, tag="z", name="z")
            zeng = nc.vector if (tail and jj >= 12 and jj % 2 == 0) else nc.gpsimd
            zeng.tensor_scalar(
                out=z,
                in0=st["Y"][:, jj, :],
                scalar1=mu_s,
                scalar2=st["rstd"][:, jj : jj + 1],
                op0=ALU.subtract,
                op1=ALU.mult,
            )
            z2 = stats.tile([128, D], f32, tag="z2", name="z2")
            zeng.tensor_mul(out=z2, in0=z, in1=gb)
            zeng.tensor_add(out=st["Yout"][:, jj, :], in0=z2, in1=bb)

        def emit_outdma(b, st, half=None, quarter=None, eng=None):
            ov = o_d[b].rearrange("(t p) d -> p t d", p=128)
            eng = eng or nc.sync
            if quarter is not None:
                q4 = slice(quarter * 4, (quarter + 1) * 4)
                eng.dma_start(out=ov[:, q4, :], in_=st["Yout"][:, q4, :])
            elif half is None:
                eng.dma_start(out=ov, in_=st["Yout"])
            else:
                h8 = slice(half * 8, (half + 1) * 8)
                eng.dma_start(out=ov[:, h8, :], in_=st["Yout"][:, h8, :])

        # ---- software-pipelined schedule over the two batches ---------------
        A, Bst = {}, {}
        emit_loads(0, A)
        emit_loads_xT(0, A)
        emit_stats_a(0, A)
        emit_stats_b(0, A)
        emit_loads(1, Bst)
        emit_loads_xT(1, Bst)
        gb = consts.tile([128, D], f32, tag="gb", name="gb")
        bb = consts.tile([128, D], f32, tag="bb", name="bb")
        for j in range(NT):
            if j in (4, 8, 12):
                emit_stats_q(0, A, j // 4)
            emit_qk_exp(0, A, j)
            if j > 0:
                emit_av(0, A, j - 1)
            if j == 8:
                with tc.tile_wait_until(0.018):
                    emit_stats_a(1, Bst)
                    emit_stats_b(1, Bst)
            if j in (10, 12, 14):
                with tc.tile_wait_until(0.018 + 0.002 * (j - 8)):
                    emit_stats_q(1, Bst, (j - 8) // 2)
            if j == 5:
                nc.sync.dma_start(out=gb, in_=g_d[:].partition_broadcast(128))
                nc.sync.dma_start(out=bb, in_=b_d[:].partition_broadcast(128))
        emit_av(0, A, NT - 1)
        emit_den(0, A)
        emit_drain(0, A)
        # phase 1: batch 1's main loop with batch 0's whole output stage
        # threaded through it (outA x2 in early iters, lnr at 8, outB x2 late)
        for j in range(NT):
            emit_qk_exp(1, Bst, j)
            if j > 0:
                emit_av(1, Bst, j - 1)
            if 3 <= j < 11:
                emit_outA(0, A, 2 * (j - 3))
                emit_outA(0, A, 2 * (j - 3) + 1)
            elif j >= 11:
                if j == 11:
                    emit_lnr(0, A)
                for q in range(3):
                    emit_outB(0, A, 3 * (j - 11) + q)
                if j == 14:
                    emit_outdma(0, A, half=0)
        emit_av(1, Bst, NT - 1)
        emit_outB(0, A, 15)
        emit_outdma(0, A, half=1, eng=nc.gpsimd)
        Bst["numS"] = Bst["num"]
        Bst["Y"] = big.tile([128, NT, D], f32, tag="Y", name="Y")
        Bst["Y1"] = big.tile([128, NT, D], f32, tag="Y1", name="Y1", bufs=1)
        Bst["Yout"] = big.tile([128, NT, D], f32, tag="Yout", name="Yout")
        Bst["MV"] = stats.tile([128, NT, 2], f32, tag="MV", name="MV")
        # quarter-pipelined tail: the ACT y1 stream leads; each quarter's
        # lnr/outB/store trails one quarter behind so no engine head-of-line
        # blocks another
        Bst["denS"] = stats.tile([128, NT], f32, tag="denS", name="denS")
        Bst["R"] = stats.tile([128, NT], f32, tag="R", name="R")
        for q in range(5):
            if q < 4:
                qs = slice(4 * q, 4 * q + 4)
                nc.vector.tensor_copy(out=Bst["denS"][:, qs], in_=Bst["denP"][:, qs])
                nc.vector.reciprocal(out=Bst["R"][:, qs], in_=Bst["denS"][:, qs])
            if q >= 1:
                with tc.high_priority():
                    emit_lnr_mv(1, Bst, lo=4 * (q - 1), hi=4 * q)
                    for jj in range(4 * (q - 1), 4 * q):
                        emit_outB(1, Bst, jj, tail=True)
                if q == 4:
                    ov = o_d[1].rearrange("(t p) d -> p t d", p=128)
                    nc.scalar.dma_start(
                        out=ov[:, 12:14, :], in_=Bst["Yout"][:, 12:14, :]
                    )
                    nc.gpsimd.dma_start(
                        out=ov[:, 14:16, :], in_=Bst["Yout"][:, 14:16, :]
                    )
                else:
                    emit_outdma(1, Bst, quarter=q - 1, eng=nc.sync)
            if q < 4:
                for jj in range(4 * q, 4 * q + 4):
                    emit_outT(1, Bst, jj)

    nc.finalize()
    return nc


def _get_nc():
    if "nc" not in _CACHE:
        _CACHE["nc"] = _build()
    return _CACHE["nc"]


def _run(x, gamma, beta, trace=False):
    import ml_dtypes

    from concourse.bass_utils import run_bass_kernel_spmd

    x = np.ascontiguousarray(np.asarray(x, dtype=np.float32))
    gamma = np.ascontiguousarray(np.asarray(gamma, dtype=np.float32))
    beta = np.ascontiguousarray(np.asarray(beta, dtype=np.float32))

    xs = x.reshape(N_CORES, NB, T, D)
    xbf = xs.astype(ml_dtypes.bfloat16)
    xTs = np.ascontiguousarray(xs.transpose(0, 1, 3, 2)).astype(ml_dtypes.bfloat16)

    in_maps = [
        {
            "x": np.ascontiguousarray(xbf[c]),
            "xT": xTs[c],
            "gamma": gamma,
            "beta": beta,
        }
        for c in range(N_CORES)
    ]
    res = run_bass_kernel_spmd(
        _get_nc(), in_maps, core_ids=list(range(N_CORES)), trace=trace
    )
    out = np.stack([res.results[c]["out"] for c in range(N_CORES)], axis=0)
    return out.reshape(B, T, D), res


def kernel(x, gamma, beta):
    out, _ = _run(x, gamma, beta, trace=False)
    return out


# revision 66
# speedup vs baseline: 1.0099x; 1.0099x over previous
"""Fused self-attention + residual + LayerNorm kernel for Trainium2.

Reference computation (per batch b of 16):
    S    = x @ x.T                  [2048, 2048]
    A    = softmax(S, axis=-1)
    out  = A @ x                    [2048, 128]
    y    = out + x
    res  = LayerNorm(y) * gamma + beta
Sharding: data-parallel over batch, 2 batches per core on 8 NeuronCores (SPMD,
no collectives).

Algorithm notes (per core / per batch):
  * Stabilized softmax without a max pass: P[q,k] = exp(S[q,k] - (c_q+G)/2)
    with c_q = ||x_q||^2 and G a mid-range constant (soft-max/soft-min of c
    via exp(+-c/6) sums + float-bit ln).  Cauchy-Schwarz bounds the exponent
    by (c_k - G)/2, safely inside bf16/f32 range for G mid-range.
  * No explicit transpose or symmetrization anywhere: the AV matmul uses the
    q-major P tiles directly as lhsT.  Since S is symmetric, tile (j,jj) of P
    equals exp(S[q',k] - (c_k+G)/2) for q' in block jj, k in block j -- so
    with scaled values Vt[k] = t_k x[k] (t_k = exp((c_k-G)/2)) the per-k
    factors cancel: the accumulated result is num'[q'] = t_q' * num[q'].
    The stray t_q' folds into the existing per-row normalization scalar
    R = 1/(den_q * t_q) -- the elementwise P->E multiply of the symmetric
    formulation disappears entirely.
  * exp runs on ACT (bias per-partition, accum_out = row-sum denominators
    for free) for most j-steps; a subset of steps instead computes P on the
    gpsimd engine with a two-op Schraudolph bit-trick
        w = max(S + bias_q, -88);  P_bits(int16) = trunc(A16*w + B16)
    bitcast to bf16 (A16 = 128/ln2).  The clamp keeps the int16 conversion
    out of the NaN band; softmax normalization cancels the ~3% per-entry
    error.  Denominators for those steps come from a DVE row-reduce of P.
    This splits the exp workload across ACT/Pool/DVE so the tensor engine
    (QK^T + AV at their streaming rooflines) becomes the bottleneck.
  * QK^T and AV run in bf16 (f32 PSUM accumulation); rsqrt for LayerNorm is
    fast-inverse-sqrt + 2 Newton steps so ACT stays on one table set.
  * The two batches are software-pipelined: batch 1's main loop overlaps
    batch 0's output stage, and each engine's issue order is time-monotone.
"""

import sys

import numpy as np

sys.path.insert(0, "/opt/trn_rl_repo")

B, T, D = 16, 2048, 128
N_CORES = 8
NB = B // N_CORES          # batches per core
NT = T // 128              # 128-row tiles per batch
EPS = 1e-5

# per-slab engine for the 64 512-wide exp slabs (slab s = 4*j + h):
# 'A' = ACT LUT exp, 'P' = gpsimd two-op Schraudolph.  Pattern [A,P,A]
# repeating = 43 ACT + 21 Pool, so ACT and Pool consume adjacent S slabs
# concurrently instead of starving each other.  Denominators cost nothing
# here: they accumulate on the PE as N=1 matmuls (rhs = t column) reusing
# the AV matmuls' stationary weights, into a dedicated PSUM bank.
MODE = {s: ("P" if (s % 3 == 2 and s >= 2) else "A") for s in range(64)}
A16 = 128.0 / 0.6931471805599453        # bf16 Schraudolph scale
B16 = 16251.0                           # 127*128 - minimax shift + trunc comp
LN2 = 0.6931471805599453

_CACHE = {}


def _build():
    from contextlib import ExitStack

    import concourse.bacc as bacc
    import concourse.bass as bass  # noqa: F401
    import concourse.tile as tile
    from concourse import mybir

    f32 = mybir.dt.float32
    bf = mybir.dt.bfloat16
    i16 = mybir.dt.int16
    AF = mybir.ActivationFunctionType
    ALU = mybir.AluOpType
    AX = mybir.AxisListType

    nc = bacc.Bacc()

    x_d = nc.dram_tensor("x", [NB, T, D], bf, kind="ExternalInput")
    xT_d = nc.dram_tensor("xT", [NB, D, T], bf, kind="ExternalInput")
    g_d = nc.dram_tensor("gamma", [D], f32, kind="ExternalInput")
    b_d = nc.dram_tensor("beta", [D], f32, kind="ExternalInput")
    o_d = nc.dram_tensor("out", [NB, T, D], f32, kind="ExternalOutput")

    ctx = ExitStack()
    with tile.TileContext(nc) as tc, ctx:
        big = ctx.enter_context(tc.tile_pool(name="big", bufs=2))
        epool = ctx.enter_context(tc.tile_pool(name="epool", bufs=12))
        stats = ctx.enter_context(tc.tile_pool(name="stats", bufs=2))
        consts = ctx.enter_context(tc.tile_pool(name="consts", bufs=1))
        spool = ctx.enter_context(tc.tile_pool(name="spool", bufs=3, space="PSUM"))
        npool = ctx.enter_context(tc.tile_pool(name="npool", bufs=1, space="PSUM"))
        dpool = ctx.enter_context(tc.tile_pool(name="dpool", bufs=1, space="PSUM"))

        zero_t = consts.tile([128, 1], f32, tag="zero", name="zero")
        nc.vector.memset(zero_t, 0.0)
        ones_c = consts.tile([128, 1], f32, tag="ones_c", name="ones_c")
        nc.vector.memset(ones_c, 1.0)
        ones_r = consts.tile([1, 128], f32, tag="ones_r", name="ones_r")
        nc.vector.memset(ones_r, 1.0)

        def emit_loads(b, st):
            st["xT"] = big.tile([128, T], bf, tag="xT", name="xT")
            st["x"] = big.tile([128, NT, D], bf, tag="x", name="x")
            xv = x_d[b].rearrange("(t p) d -> p t d", p=128)
            for sx in range(4):
                nc.sync.dma_start(
                    out=st["x"][:, sx * 4 : (sx + 1) * 4, :],
                    in_=xv[:, sx * 4 : (sx + 1) * 4, :],
                )

        def emit_loads_xT(b, st):
            # b0: halves on the ACT and Pool DGE queues so the Pool sqb
            # chain only waits for one half
            engs = (nc.scalar, nc.gpsimd) if b == 0 else (nc.sync, nc.sync)
            for sx in range(2):
                engs[sx].dma_start(
                    out=st["xT"][:, sx * 1024 : (sx + 1) * 1024],
                    in_=xT_d[b, :, sx * 1024 : (sx + 1) * 1024],
                )

        def emit_stats_a(b, st):
            # x row-norms: gpsimd squares; the per-slice DVE reduces are in
            # emit_stats_b's quarter loop so wait-hints keep them off the
            # G-chain's critical path
            x_sb = st["x"]
            st["C"] = stats.tile([128, NT], f32, tag="C", name="C")
            st["bias"] = stats.tile([128, NT], f32, tag="bias", name="bias")
            st["sqb"] = sqb = big.tile([128, NT, D], f32, tag="sqb", name="sqb")
            for t in range(NT):
                nc.gpsimd.tensor_mul(
                    out=sqb[:, t, :], in0=x_sb[:, t, :], in1=x_sb[:, t, :]
                )
            nc.vector.tensor_reduce(
                out=st["C"][:, 0:4], in_=sqb[:, 0:4, :], axis=AX.X, op=ALU.add
            )

        def emit_stats_b(b, st):
            # Soft bounds on the range of c without cross-partition reductions:
            #   cbar ~ 6 ln(sum exp(c/6)),  mbar ~ -6 ln(sum exp(-c/6))
            # over the FIRST 4 row-tiles only (512 rows bound the range within
            # a few units -- the +-75 exponent margins absorb that), so the
            # G chain starts after the first x DMA slice.  The ln's use
            # Schraudolph float-bits; cross-partition sum and the broadcast
            # back are K=1/M=1,2 matmuls on the PE into the den PSUM bank.
            C = st["C"]
            ec2 = stats.tile([128, 2], f32, tag="ec2", name="ec2")
            EC = stats.tile([128, 4], f32, tag="EC", name="EC")
            nc.scalar.activation(
                out=EC, in_=C[:, 0:4], func=AF.Exp, bias=zero_t, scale=1.0 / 6.0,
                accum_out=ec2[:, 0:1],
            )
            ECm = stats.tile([128, 4], f32, tag="ECm", name="ECm")
            nc.scalar.activation(
                out=ECm, in_=C[:, 0:4], func=AF.Exp, bias=zero_t, scale=-1.0 / 6.0,
                accum_out=ec2[:, 1:2],
            )
            s1a = spool.tile([1, 1], f32, tag="S", name="s1a")
            nc.tensor.matmul(out=s1a, lhsT=ec2[:, 0:1], rhs=ones_c, start=True, stop=True)
            s1b = spool.tile([1, 1], f32, tag="S", name="s1b")
            nc.tensor.matmul(out=s1b, lhsT=ec2[:, 1:2], rhs=ones_c, start=True, stop=True)
            LL2 = stats.tile([1, 2], f32, tag="LL2", name="LL2")
            nc.vector.tensor_copy(out=LL2[0:1, 0:1], in_=s1a.bitcast(mybir.dt.int32))
            nc.vector.tensor_copy(out=LL2[0:1, 1:2], in_=s1b.bitcast(mybir.dt.int32))
            s2 = spool.tile([128, 2], f32, tag="S", name="s2")
            nc.tensor.matmul(out=s2, lhsT=ones_r, rhs=LL2, start=True, stop=True)
            # -G/2 = (3*LN2/2^24)*(bits_minus - bits_plus)  [128,1]
            s2s = stats.tile([128, 2], f32, tag="s2s", name="s2s")
            nc.vector.tensor_copy(out=s2s, in_=s2)
            Gd = stats.tile([128, 1], f32, tag="Gd", name="Gd")
            nc.vector.tensor_tensor(
                out=Gd, in0=s2s[:, 1:2], in1=s2s[:, 0:1], op=ALU.subtract
            )
            Gh_neg = stats.tile([128, 1], f32, tag="Ghn", name="Ghn")
            nc.vector.tensor_scalar_mul(out=Gh_neg, in0=Gd, scalar1=1.5 * LN2 / 8388608.0)
            # per-quarter: remaining C reduces (wait-hinted off the critical
            # path), bias chunk, t_k = exp((c_k-G)/2), Vt = t*x, Tb = bf16 t.
            # The first quarter gates the first exps/AVs; later quarters only
            # need to beat their own j-steps.
            st["Ghn"] = Gh_neg
            st["Tall"] = stats.tile([128, NT], f32, tag="Tall", name="Tall")
            st["Vt"] = big.tile([128, NT, D], bf, tag="Vt", name="Vt")
            st["Tb"] = stats.tile([128, NT], bf, tag="Tb", name="Tb")
            emit_stats_q(b, st, 0)

        def emit_stats_q(b, st, qd):
            # quarter qd of: C reduce, bias chunk, t = exp((c-G)/2), bf16 t,
            # Vt = t*x.  Emitted right before j-step 4*qd uses them, so their
            # emission priority can't preempt the head's critical chain.
            C, Gh_neg, Tall = st["C"], st["Ghn"], st["Tall"]
            cs = slice(4 * qd, 4 * qd + 4)
            if qd > 0:
                nc.vector.tensor_reduce(
                    out=C[:, cs], in_=st["sqb"][:, cs, :], axis=AX.X, op=ALU.add
                )
            nc.vector.tensor_scalar(
                out=st["bias"][:, cs],
                in0=C[:, cs],
                scalar1=-0.5,
                scalar2=Gh_neg,
                op0=ALU.mult,
                op1=ALU.add,
            )
            nc.scalar.activation(
                out=Tall[:, cs], in_=C[:, cs], func=AF.Exp, bias=Gh_neg, scale=0.5
            )
            nc.vector.tensor_copy(out=st["Tb"][:, cs], in_=Tall[:, cs])
            for t in range(4 * qd, 4 * qd + 4):
                nc.gpsimd.tensor_scalar_mul(
                    out=st["Vt"][:, t, :], in0=st["x"][:, t, :],
                    scalar1=Tall[:, t : t + 1],
                )

        def emit_qk_exp(b, st, j):
            # QK^T for row-block j + exp into four bf16 quarter-tiles.  The AV
            # matmuls for block j are emitted one step later (emit_av) so PE
            # work overlaps the exp of the next block instead of gating it.
            if j == 0:
                st["num"] = npool.tile([128, T], f32, tag="num", name="num")
                st["denP"] = dpool.tile([128, NT], f32, tag="denP", name="denP")
                st["E"] = {}
            Eq = [
                epool.tile([128, 512], bf, tag="E", name="E") for _ in range(4)
            ]
            st["E"][j] = Eq
            xT_sb = st["xT"]
            for h in range(4):
                S = spool.tile([128, 512], f32, tag="S", name="S")
                nc.tensor.matmul(
                    out=S,
                    lhsT=xT_sb[:, j * 128 : (j + 1) * 128],
                    rhs=xT_sb[:, h * 512 : (h + 1) * 512],
                    start=True,
                    stop=True,
                )
                if MODE[4 * j + h] == "P":
                    # DVE+Pool 2-op Schraudolph: w = max(S+bias,-88) on DVE
                    # (gpsimd cannot read PSUM); bits = trunc(A16*w + B16) on
                    # gpsimd -> int16 view of bf16 tile
                    tmp = big.tile([128, 512], f32, tag="stmp", name="stmp", bufs=3)
                    with tc.high_priority():
                        nc.vector.tensor_scalar(
                            out=tmp,
                            in0=S,
                            scalar1=st["bias"][:, j : j + 1],
                            scalar2=-88.0,
                            op0=ALU.add,
                            op1=ALU.max,
                        )
                    nc.gpsimd.tensor_scalar(
                        out=Eq[h].bitcast(i16),
                        in0=tmp,
                        scalar1=A16,
                        scalar2=B16,
                        op0=ALU.mult,
                        op1=ALU.add,
                    )
                else:
                    nc.scalar.activation(
                        out=Eq[h],
                        in_=S,
                        func=AF.Exp,
                        bias=st["bias"][:, j : j + 1],
                        scale=1.0,
                    )

        def emit_av(b, st, j):
            Eq = st["E"].pop(j)
            for jj in range(NT):
                lhsT = Eq[jj // 4][:, (jj % 4) * 128 : (jj % 4 + 1) * 128]
                # 4 output slices share a 2KB PSUM bank = one zero region:
                # only the bank's first MM sets start, only its last sets stop
                nc.tensor.matmul(
                    out=st["num"][:, jj * 128 : (jj + 1) * 128],
                    lhsT=lhsT,
                    rhs=st["Vt"][:, j, :],
                    start=(j == 0 and jj % 4 == 0),
                    stop=(j == NT - 1 and jj % 4 == 3),
                )
                # denominator: same stationary weights, t column as rhs ->
                # denP[q', jj] accumulates sum_k exp(S[q',k]-G) over all j
                nc.tensor.matmul(
                    out=st["denP"][:, jj : jj + 1],
                    lhsT=lhsT,
                    rhs=st["Tb"][:, j : j + 1],
                    start=(j == 0 and jj == 0),
                    stop=(j == NT - 1 and jj == NT - 1),
                )

        def emit_den(b, st):
            denS = stats.tile([128, NT], f32, tag="denS", name="denS")
            nc.vector.tensor_copy(out=denS, in_=st["denP"])
            R = stats.tile([128, NT], f32, tag="R", name="R")
            nc.vector.reciprocal(out=R, in_=denS)
            st["R"] = R

        def emit_drain(b, st, copy_psum=True, half=None):
            # drain AV results out of PSUM so the next batch can reuse it
            # (skipped for the last batch -- nothing needs the banks).
            # Staggered: half 0 at the phase boundary, half 1 a few iterations
            # later, so the copies don't starve DVE mid-phase.
            if copy_psum:
                if half != 1:
                    st["numS"] = big.tile([128, T], f32, tag="numS", name="numS")
                for h in ([0, 1] if half is None else [half]):
                    if h == 0:
                        nc.vector.tensor_copy(
                            out=st["numS"][:, 0:1024],
                            in_=st["num"][:, 0:1024],
                        )
                    else:
                        nc.scalar.activation(
                            out=st["numS"][:, 1024:2048],
                            in_=st["num"][:, 1024:2048],
                            func=AF.Copy,
                        )
            else:
                st["numS"] = st["num"]
            if half != 1:
                st["Y"] = big.tile([128, NT, D], f32, tag="Y", name="Y")
                st["Yout"] = big.tile([128, NT, D], f32, tag="Yout", name="Yout")
                st["Sy"] = stats.tile([128, NT], f32, tag="Sy", name="Sy")
                st["Sy2"] = stats.tile([128, NT], f32, tag="Sy2", name="Sy2")
                st["ysqb"] = big.tile([128, NT, D], f32, tag="ysqb", name="ysqb")

        def emit_outA(b, st, jj, tail=False):
            # y = num'/den' + x, with LN stats via accum_out.  The y-compute
            # reads numS (PSUM for the last batch) so it goes on DVE unless
            # numS was drained to SBUF, in which case odd tiles go on gpsimd.
            nc.vector.scalar_tensor_tensor(
                out=st["Y"][:, jj, :],
                in0=st["numS"][:, jj * 128 : (jj + 1) * 128],
                scalar=st["R"][:, jj : jj + 1],
                in1=st["x"][:, jj, :],
                op0=ALU.mult,
                op1=ALU.add,
                accum_out=st["Sy"][:, jj : jj + 1],
            )
            if tail and jj % 2 == 0:
                nc.scalar.activation(
                    out=st["ysqb"][:, jj, :],
                    in_=st["Y"][:, jj, :],
                    func=AF.Square,
                    accum_out=st["Sy2"][:, jj : jj + 1],
                )
            else:
                nc.vector.scalar_tensor_tensor(
                    out=st["ysqb"][:, jj, :],
                    in0=st["Y"][:, jj, :],
                    scalar=1.0,
                    in1=st["Y"][:, jj, :],
                    op0=ALU.mult,
                    op1=ALU.mult,
                    accum_out=st["Sy2"][:, jj : jj + 1],
                )

        def emit_lnr(b, st, lo=0, hi=NT):
            cs = slice(lo, hi)
            if True:
                if "mu" not in st:
                    st["mu"] = stats.tile([128, NT], f32, tag="mu", name="mu")
                    st["vart"] = stats.tile([128, NT], f32, tag="vart", name="vart")
                    st["rstd"] = stats.tile([128, NT], f32, tag="rstd", name="rstd")
                # mu = Sy/128, var = Sy2/128 - mu^2
                nc.vector.tensor_scalar_mul(
                    out=st["mu"][:, cs], in0=st["Sy"][:, cs], scalar1=1.0 / D
                )
                musq = stats.tile([128, NT], f32, tag="musq", name="musq")
                nc.vector.scalar_tensor_tensor(
                    out=musq[:, cs],
                    in0=st["mu"][:, cs],
                    scalar=1.0,
                    in1=st["mu"][:, cs],
                    op0=ALU.mult,
                    op1=ALU.mult,
                )
                nc.vector.scalar_tensor_tensor(
                    out=st["vart"][:, cs],
                    in0=st["Sy2"][:, cs],
                    scalar=1.0 / D,
                    in1=musq[:, cs],
                    op0=ALU.mult,
                    op1=ALU.subtract,
                )
                var_in = st["vart"][:, cs]
            # rstd = 1/sqrt(var+eps) via the fast-inverse-sqrt bit trick plus
            # two Newton steps (~4e-6 rel err) -- keeps the ACT engine on the
            # exp table set for the whole kernel (table swaps cost 1.3us each)
            ve = stats.tile([128, NT], f32, tag="ve", name="ve")
            nc.vector.tensor_scalar_add(out=ve[:, cs], in0=var_in, scalar1=EPS)
            wf = stats.tile([128, NT], f32, tag="wf", name="wf")
            nc.vector.tensor_copy(out=wf[:, cs], in_=ve[:, cs].bitcast(mybir.dt.int32))
            nc.vector.tensor_scalar(
                out=wf[:, cs], in0=wf[:, cs],
                scalar1=-0.5, scalar2=1597463007.0,
                op0=ALU.mult, op1=ALU.add,
            )
            wi = stats.tile([128, NT], mybir.dt.int32, tag="wi", name="wi")
            nc.vector.tensor_copy(out=wi[:, cs], in_=wf[:, cs])
            y = stats.tile([128, NT], f32, tag="y0", name="y0")
            nc.vector.tensor_copy(out=y[:, cs], in_=wi[:, cs].bitcast(f32))
            t1 = stats.tile([128, NT], f32, tag="t1", name="t1")
            for it in range(2):
                nc.vector.tensor_mul(out=t1[:, cs], in0=ve[:, cs], in1=y[:, cs])
                nc.vector.tensor_mul(out=t1[:, cs], in0=t1[:, cs], in1=y[:, cs])
                nc.vector.tensor_scalar(
                    out=t1[:, cs], in0=t1[:, cs],
                    scalar1=-0.5, scalar2=1.5, op0=ALU.mult, op1=ALU.add,
                )
                nc.vector.tensor_mul(
                    out=st["rstd"][:, cs] if it == 1 else y[:, cs],
                    in0=y[:, cs], in1=t1[:, cs],
                )

        def emit_outT(b, st, jj):
            # tail path: ACT scales num by R (idle engine there), Pool adds the
            # residual, DVE computes LN stats via bn_stats/bn_aggr
            nc.scalar.activation(
                out=st["Y1"][:, jj, :],
                in_=st["num"][:, jj * 128 : (jj + 1) * 128],
                func=AF.Copy,
                scale=st["R"][:, jj : jj + 1],
            )
            nc.gpsimd.tensor_add(
                out=st["Y"][:, jj, :], in0=st["Y1"][:, jj, :], in1=st["x"][:, jj, :]
            )
            bns = stats.tile([128, 6], f32, tag="bns", name="bns")
            nc.vector.bn_stats(out=bns, in_=st["Y"][:, jj, :])
            nc.vector.bn_aggr(out=st["MV"][:, jj, :], in_=bns)

        def emit_lnr_mv(b, st, lo=0, hi=NT):
            cs = slice(lo, hi)
            if "rstd" not in st:
                st["rstd"] = stats.tile([128, NT], f32, tag="rstd", name="rstd")
                st["ve"] = stats.tile([128, NT], f32, tag="ve", name="ve")
                st["wf"] = stats.tile([128, NT], f32, tag="wf", name="wf")
                st["wi"] = stats.tile([128, NT], mybir.dt.int32, tag="wi", name="wi")
                st["y0"] = stats.tile([128, NT], f32, tag="y0", name="y0")
                st["t1"] = stats.tile([128, NT], f32, tag="t1", name="t1")
            ve = st["ve"]
            nc.vector.tensor_scalar_add(out=ve[:, cs], in0=st["MV"][:, cs, 1], scalar1=EPS)
            wf = st["wf"]
            nc.vector.tensor_copy(out=wf[:, cs], in_=ve[:, cs].bitcast(mybir.dt.int32))
            nc.vector.tensor_scalar(
                out=wf[:, cs], in0=wf[:, cs],
                scalar1=-0.5, scalar2=1597463007.0,
                op0=ALU.mult, op1=ALU.add,
            )
            wi = st["wi"]
            nc.vector.tensor_copy(out=wi[:, cs], in_=wf[:, cs])
            y = st["y0"]
            nc.vector.tensor_copy(out=y[:, cs], in_=wi[:, cs].bitcast(f32))
            t1 = st["t1"]
            for it in range(2):
                nc.vector.tensor_mul(out=t1[:, cs], in0=ve[:, cs], in1=y[:, cs])
                nc.vector.tensor_mul(out=t1[:, cs], in0=t1[:, cs], in1=y[:, cs])
                nc.vector.tensor_scalar(
                    out=t1[:, cs], in0=t1[:, cs],
                    scalar1=-0.5, scalar2=1.5, op0=ALU.mult, op1=ALU.add,
                )
                nc.vector.tensor_mul(
                    out=st["rstd"][:, cs] if it == 1 else y[:, cs],
                    in0=y[:, cs], in1=t1[:, cs],
                )

        def emit_outB(b, st, jj, tail=False):
            mu_s = st["MV"][:, jj, 0:1] if "MV" in st else st["mu"][:, jj : jj + 1]
            z = stats.tile([128, D], f32, tag="z", name="z")
            zeng = nc.vector if (tail and jj >= 12 and jj % 2 == 0) else nc.gpsimd
            zeng.tensor_scalar(
                out=z,
                in0=st["Y"][:, jj, :],
                scalar1=mu_s,
                scalar2=st["rstd"][:, jj : jj + 1],
                op0=ALU.subtract,
                op1=ALU.mult,
            )
            z2 = stats.tile([128, D], f32, tag="z2", name="z2")
            zeng.tensor_mul(out=z2, in0=z, in1=gb)
            zeng.tensor_add(out=st["Yout"][:, jj, :], in0=z2, in1=bb)

        def emit_outdma(b, st, half=None, quarter=None, eng=None):
            ov = o_d[b].rearrange("(t p) d -> p t d", p=128)
            eng = eng or nc.sync
            if quarter is not None:
                q4 = slice(quarter * 4, (quarter + 1) * 4)
                eng.dma_start(out=ov[:, q4, :], in_=st["Yout"][:, q4, :])
            elif half is None:
                eng.dma_start(out=ov, in_=st["Yout"])
            else:
                h8 = slice(half * 8, (half + 1) * 8)
                eng.dma_start(out=ov[:, h8, :], in_=st["Yout"][:, h8, :])

        # ---- software-pipelined schedule over the two batches ---------------
        A, Bst = {}, {}
        emit_loads(0, A)
        emit_loads_xT(0, A)
        emit_stats_a(0, A)
        emit_stats_b(0, A)
        emit_loads(1, Bst)
        emit_loads_xT(1, Bst)
        gb = consts.tile([128, D], f32, tag="gb", name="gb")
        bb = consts.tile([128, D], f32, tag="bb", name="bb")
        for j in range(NT):
            if j in (4, 8, 12):
                emit_stats_q(0, A, j // 4)
            emit_qk_exp(0, A, j)
            if j > 0:
                emit_av(0, A, j - 1)
            if j == 8:
                with tc.tile_wait_until(0.018):
                    emit_stats_a(1, Bst)
                    emit_stats_b(1, Bst)
            if j in (10, 12, 14):
                with tc.tile_wait_until(0.018 + 0.002 * (j - 8)):
                    emit_stats_q(1, Bst, (j - 8) // 2)
            if j == 5:
                nc.sync.dma_start(out=gb, in_=g_d[:].partition_broadcast(128))
                nc.sync.dma_start(out=bb, in_=b_d[:].partition_broadcast(128))
        emit_av(0, A, NT - 1)
        emit_den(0, A)
        emit_drain(0, A)
        # phase 1: batch 1's main loop with batch 0's whole output stage
        # threaded through it (outA x2 in early iters, lnr at 8, outB x2 late)
        for j in range(NT):
            emit_qk_exp(1, Bst, j)
            if j > 0:
                emit_av(1, Bst, j - 1)
            if 3 <= j < 11:
                emit_outA(0, A, 2 * (j - 3))
                emit_outA(0, A, 2 * (j - 3) + 1)
            elif j >= 11:
                if j == 11:
                    emit_lnr(0, A)
                for q in range(3):
                    emit_outB(0, A, 3 * (j - 11) + q)
                if j == 14:
                    emit_outdma(0, A, half=0)
        emit_av(1, Bst, NT - 1)
        emit_outB(0, A, 15)
        emit_outdma(0, A, half=1)
        Bst["numS"] = Bst["num"]
        Bst["Y"] = big.tile([128, NT, D], f32, tag="Y", name="Y")
        Bst["Y1"] = big.tile([128, NT, D], f32, tag="Y1", name="Y1", bufs=1)
        Bst["Yout"] = big.tile([128, NT, D], f32, tag="Yout", name="Yout")
        Bst["MV"] = stats.tile([128, NT, 2], f32, tag="MV", name="MV")
        # quarter-pipelined tail: the ACT y1 stream leads; each quarter's
        # lnr/outB/store trails one quarter behind so no engine head-of-line
        # blocks another
        Bst["denS"] = stats.tile([128, NT], f32, tag="denS", name="denS")
        Bst["R"] = stats.tile([128, NT], f32, tag="R", name="R")
        for q in range(5):
            if q < 4:
                qs = slice(4 * q, 4 * q + 4)
                nc.vector.tensor_copy(out=Bst["denS"][:, qs], in_=Bst["denP"][:, qs])
                nc.vector.reciprocal(out=Bst["R"][:, qs], in_=Bst["denS"][:, qs])
            if q >= 1:
                with tc.high_priority():
                    emit_lnr_mv(1, Bst, lo=4 * (q - 1), hi=4 * q)
                    for jj in range(4 * (q - 1), 4 * q):
                        emit_outB(1, Bst, jj, tail=True)
                if q == 4:
                    ov = o_d[1].rearrange("(t p) d -> p t d", p=128)
                    nc.scalar.dma_start(
                        out=ov[:, 12:14, :], in_=Bst["Yout"][:, 12:14, :]
                    )
                    nc.gpsimd.dma_start(
                        out=ov[:, 14:16, :], in_=Bst["Yout"][:, 14:16, :]
                    )
                else:
                    emit_outdma(1, Bst, quarter=q - 1, eng=nc.sync)
            if q < 4:
                for jj in range(4 * q, 4 * q + 4):
                    emit_outT(1, Bst, jj)

    nc.finalize()
    return nc


def _get_nc():
    if "nc" not in _CACHE:
        _CACHE["nc"] = _build()
    return _CACHE["nc"]


def _run(x, gamma, beta, trace=False):
    import ml_dtypes

    from concourse.bass_utils import run_bass_kernel_spmd

    x = np.ascontiguousarray(np.asarray(x, dtype=np.float32))
    gamma = np.ascontiguousarray(np.asarray(gamma, dtype=np.float32))
    beta = np.ascontiguousarray(np.asarray(beta, dtype=np.float32))

    xs = x.reshape(N_CORES, NB, T, D)
    xbf = xs.astype(ml_dtypes.bfloat16)
    xTs = np.ascontiguousarray(xs.transpose(0, 1, 3, 2)).astype(ml_dtypes.bfloat16)

    in_maps = [
        {
            "x": np.ascontiguousarray(xbf[c]),
            "xT": xTs[c],
            "gamma": gamma,
            "beta": beta,
        }
        for c in range(N_CORES)
    ]
    res = run_bass_kernel_spmd(
        _get_nc(), in_maps, core_ids=list(range(N_CORES)), trace=trace
    )
    out = np.stack([res.results[c]["out"] for c in range(N_CORES)], axis=0)
    return out.reshape(B, T, D), res


def kernel(x, gamma, beta):
    out, _ = _run(x, gamma, beta, trace=False)
    return out


# revision 67
# speedup vs baseline: 1.0107x; 1.0008x over previous
"""Fused self-attention + residual + LayerNorm kernel for Trainium2.

Reference computation (per batch b of 16):
    S    = x @ x.T                  [2048, 2048]
    A    = softmax(S, axis=-1)
    out  = A @ x                    [2048, 128]
    y    = out + x
    res  = LayerNorm(y) * gamma + beta
Sharding: data-parallel over batch, 2 batches per core on 8 NeuronCores (SPMD,
no collectives).

Algorithm notes (per core / per batch):
  * Stabilized softmax without a max pass: P[q,k] = exp(S[q,k] - (c_q+G)/2)
    with c_q = ||x_q||^2 and G a mid-range constant (soft-max/soft-min of c
    via exp(+-c/6) sums + float-bit ln).  Cauchy-Schwarz bounds the exponent
    by (c_k - G)/2, safely inside bf16/f32 range for G mid-range.
  * No explicit transpose or symmetrization anywhere: the AV matmul uses the
    q-major P tiles directly as lhsT.  Since S is symmetric, tile (j,jj) of P
    equals exp(S[q',k] - (c_k+G)/2) for q' in block jj, k in block j -- so
    with scaled values Vt[k] = t_k x[k] (t_k = exp((c_k-G)/2)) the per-k
    factors cancel: the accumulated result is num'[q'] = t_q' * num[q'].
    The stray t_q' folds into the existing per-row normalization scalar
    R = 1/(den_q * t_q) -- the elementwise P->E multiply of the symmetric
    formulation disappears entirely.
  * exp runs on ACT (bias per-partition, accum_out = row-sum denominators
    for free) for most j-steps; a subset of steps instead computes P on the
    gpsimd engine with a two-op Schraudolph bit-trick
        w = max(S + bias_q, -88);  P_bits(int16) = trunc(A16*w + B16)
    bitcast to bf16 (A16 = 128/ln2).  The clamp keeps the int16 conversion
    out of the NaN band; softmax normalization cancels the ~3% per-entry
    error.  Denominators for those steps come from a DVE row-reduce of P.
    This splits the exp workload across ACT/Pool/DVE so the tensor engine
    (QK^T + AV at their streaming rooflines) becomes the bottleneck.
  * QK^T and AV run in bf16 (f32 PSUM accumulation); rsqrt for LayerNorm is
    fast-inverse-sqrt + 2 Newton steps so ACT stays on one table set.
  * The two batches are software-pipelined: batch 1's main loop overlaps
    batch 0's output stage, and each engine's issue order is time-monotone.
"""

import sys

import numpy as np

sys.path.insert(0, "/opt/trn_rl_repo")

B, T, D = 16, 2048, 128
N_CORES = 8
NB = B // N_CORES          # batches per core
NT = T // 128              # 128-row tiles per batch
EPS = 1e-5

# per-slab engine for the 64 512-wide exp slabs (slab s = 4*j + h):
# 'A' = ACT LUT exp, 'P' = gpsimd two-op Schraudolph.  Pattern [A,P,A]
# repeating = 43 ACT + 21 Pool, so ACT and Pool consume adjacent S slabs
# concurrently instead of starving each other.  Denominators cost nothing
# here: they accumulate on the PE as N=1 matmuls (rhs = t column) reusing
# the AV matmuls' stationary weights, into a dedicated PSUM bank.
MODE = {s: ("P" if (s % 3 == 2 and s >= 2) else "A") for s in range(64)}
A16 = 128.0 / 0.6931471805599453        # bf16 Schraudolph scale
B16 = 16251.0                           # 127*128 - minimax shift + trunc comp
LN2 = 0.6931471805599453

_CACHE = {}


def _build():
    from contextlib import ExitStack

    import concourse.bacc as bacc
    import concourse.bass as bass  # noqa: F401
    import concourse.tile as tile
    from concourse import mybir

    f32 = mybir.dt.float32
    bf = mybir.dt.bfloat16
    i16 = mybir.dt.int16
    AF = mybir.ActivationFunctionType
    ALU = mybir.AluOpType
    AX = mybir.AxisListType

    nc = bacc.Bacc()

    x_d = nc.dram_tensor("x", [NB, T, D], bf, kind="ExternalInput")
    xT_d = nc.dram_tensor("xT", [NB, D, T], bf, kind="ExternalInput")
    g_d = nc.dram_tensor("gamma", [D], f32, kind="ExternalInput")
    b_d = nc.dram_tensor("beta", [D], f32, kind="ExternalInput")
    o_d = nc.dram_tensor("out", [NB, T, D], f32, kind="ExternalOutput")

    ctx = ExitStack()
    with tile.TileContext(nc) as tc, ctx:
        big = ctx.enter_context(tc.tile_pool(name="big", bufs=2))
        epool = ctx.enter_context(tc.tile_pool(name="epool", bufs=12))
        stats = ctx.enter_context(tc.tile_pool(name="stats", bufs=2))
        consts = ctx.enter_context(tc.tile_pool(name="consts", bufs=1))
        spool = ctx.enter_context(tc.tile_pool(name="spool", bufs=3, space="PSUM"))
        npool = ctx.enter_context(tc.tile_pool(name="npool", bufs=1, space="PSUM"))
        dpool = ctx.enter_context(tc.tile_pool(name="dpool", bufs=1, space="PSUM"))

        zero_t = consts.tile([128, 1], f32, tag="zero", name="zero")
        nc.vector.memset(zero_t, 0.0)
        ones_c = consts.tile([128, 1], f32, tag="ones_c", name="ones_c")
        nc.vector.memset(ones_c, 1.0)
        ones_r = consts.tile([1, 128], f32, tag="ones_r", name="ones_r")
        nc.vector.memset(ones_r, 1.0)

        def emit_loads(b, st):
            st["xT"] = big.tile([128, T], bf, tag="xT", name="xT")
            st["x"] = big.tile([128, NT, D], bf, tag="x", name="x")
            xv = x_d[b].rearrange("(t p) d -> p t d", p=128)
            for sx in range(4):
                nc.sync.dma_start(
                    out=st["x"][:, sx * 4 : (sx + 1) * 4, :],
                    in_=xv[:, sx * 4 : (sx + 1) * 4, :],
                )

        def emit_loads_xT(b, st):
            # b0: halves on the ACT and Pool DGE queues so the Pool sqb
            # chain only waits for one half
            engs = (nc.scalar, nc.gpsimd) if b == 0 else (nc.sync, nc.sync)
            for sx in range(2):
                engs[sx].dma_start(
                    out=st["xT"][:, sx * 1024 : (sx + 1) * 1024],
                    in_=xT_d[b, :, sx * 1024 : (sx + 1) * 1024],
                )

        def emit_stats_a(b, st):
            # x row-norms: gpsimd squares; the per-slice DVE reduces are in
            # emit_stats_b's quarter loop so wait-hints keep them off the
            # G-chain's critical path
            x_sb = st["x"]
            st["C"] = stats.tile([128, NT], f32, tag="C", name="C")
            st["bias"] = stats.tile([128, NT], f32, tag="bias", name="bias")
            st["sqb"] = sqb = big.tile([128, NT, D], f32, tag="sqb", name="sqb")
            for t in range(NT):
                nc.gpsimd.tensor_mul(
                    out=sqb[:, t, :], in0=x_sb[:, t, :], in1=x_sb[:, t, :]
                )
            nc.vector.tensor_reduce(
                out=st["C"][:, 0:4], in_=sqb[:, 0:4, :], axis=AX.X, op=ALU.add
            )

        def emit_stats_b(b, st):
            # Soft bounds on the range of c without cross-partition reductions:
            #   cbar ~ 6 ln(sum exp(c/6)),  mbar ~ -6 ln(sum exp(-c/6))
            # over the FIRST 4 row-tiles only (512 rows bound the range within
            # a few units -- the +-75 exponent margins absorb that), so the
            # G chain starts after the first x DMA slice.  The ln's use
            # Schraudolph float-bits; cross-partition sum and the broadcast
            # back are K=1/M=1,2 matmuls on the PE into the den PSUM bank.
            C = st["C"]
            ec2 = stats.tile([128, 2], f32, tag="ec2", name="ec2")
            EC = stats.tile([128, 4], f32, tag="EC", name="EC")
            nc.scalar.activation(
                out=EC, in_=C[:, 0:4], func=AF.Exp, bias=zero_t, scale=1.0 / 6.0,
                accum_out=ec2[:, 0:1],
            )
            ECm = stats.tile([128, 4], f32, tag="ECm", name="ECm")
            nc.scalar.activation(
                out=ECm, in_=C[:, 0:4], func=AF.Exp, bias=zero_t, scale=-1.0 / 6.0,
                accum_out=ec2[:, 1:2],
            )
            s1a = spool.tile([1, 1], f32, tag="S", name="s1a")
            nc.tensor.matmul(out=s1a, lhsT=ec2[:, 0:1], rhs=ones_c, start=True, stop=True)
            s1b = spool.tile([1, 1], f32, tag="S", name="s1b")
            nc.tensor.matmul(out=s1b, lhsT=ec2[:, 1:2], rhs=ones_c, start=True, stop=True)
            LL2 = stats.tile([1, 2], f32, tag="LL2", name="LL2")
            nc.vector.tensor_copy(out=LL2[0:1, 0:1], in_=s1a.bitcast(mybir.dt.int32))
            nc.vector.tensor_copy(out=LL2[0:1, 1:2], in_=s1b.bitcast(mybir.dt.int32))
            s2 = spool.tile([128, 2], f32, tag="S", name="s2")
            nc.tensor.matmul(out=s2, lhsT=ones_r, rhs=LL2, start=True, stop=True)
            # -G/2 = (3*LN2/2^24)*(bits_minus - bits_plus)  [128,1]
            s2s = stats.tile([128, 2], f32, tag="s2s", name="s2s")
            nc.vector.tensor_copy(out=s2s, in_=s2)
            Gd = stats.tile([128, 1], f32, tag="Gd", name="Gd")
            nc.vector.tensor_tensor(
                out=Gd, in0=s2s[:, 1:2], in1=s2s[:, 0:1], op=ALU.subtract
            )
            Gh_neg = stats.tile([128, 1], f32, tag="Ghn", name="Ghn")
            nc.vector.tensor_scalar_mul(out=Gh_neg, in0=Gd, scalar1=1.5 * LN2 / 8388608.0)
            # per-quarter: remaining C reduces (wait-hinted off the critical
            # path), bias chunk, t_k = exp((c_k-G)/2), Vt = t*x, Tb = bf16 t.
            # The first quarter gates the first exps/AVs; later quarters only
            # need to beat their own j-steps.
            st["Ghn"] = Gh_neg
            st["Tall"] = stats.tile([128, NT], f32, tag="Tall", name="Tall")
            st["Vt"] = big.tile([128, NT, D], bf, tag="Vt", name="Vt")
            st["Tb"] = stats.tile([128, NT], bf, tag="Tb", name="Tb")
            emit_stats_q(b, st, 0)

        def emit_stats_q(b, st, qd):
            # quarter qd of: C reduce, bias chunk, t = exp((c-G)/2), bf16 t,
            # Vt = t*x.  Emitted right before j-step 4*qd uses them, so their
            # emission priority can't preempt the head's critical chain.
            C, Gh_neg, Tall = st["C"], st["Ghn"], st["Tall"]
            cs = slice(4 * qd, 4 * qd + 4)
            if qd > 0:
                nc.vector.tensor_reduce(
                    out=C[:, cs], in_=st["sqb"][:, cs, :], axis=AX.X, op=ALU.add
                )
            nc.vector.tensor_scalar(
                out=st["bias"][:, cs],
                in0=C[:, cs],
                scalar1=-0.5,
                scalar2=Gh_neg,
                op0=ALU.mult,
                op1=ALU.add,
            )
            nc.scalar.activation(
                out=Tall[:, cs], in_=C[:, cs], func=AF.Exp, bias=Gh_neg, scale=0.5
            )
            nc.vector.tensor_copy(out=st["Tb"][:, cs], in_=Tall[:, cs])
            for t in range(4 * qd, 4 * qd + 4):
                nc.gpsimd.tensor_scalar_mul(
                    out=st["Vt"][:, t, :], in0=st["x"][:, t, :],
                    scalar1=Tall[:, t : t + 1],
                )

        def emit_qk_exp(b, st, j):
            # QK^T for row-block j + exp into four bf16 quarter-tiles.  The AV
            # matmuls for block j are emitted one step later (emit_av) so PE
            # work overlaps the exp of the next block instead of gating it.
            if j == 0:
                st["num"] = npool.tile([128, T], f32, tag="num", name="num")
                st["denP"] = dpool.tile([128, NT], f32, tag="denP", name="denP")
                st["E"] = {}
            Eq = [
                epool.tile([128, 512], bf, tag="E", name="E") for _ in range(4)
            ]
            st["E"][j] = Eq
            xT_sb = st["xT"]
            for h in range(4):
                S = spool.tile([128, 512], f32, tag="S", name="S")
                nc.tensor.matmul(
                    out=S,
                    lhsT=xT_sb[:, j * 128 : (j + 1) * 128],
                    rhs=xT_sb[:, h * 512 : (h + 1) * 512],
                    start=True,
                    stop=True,
                )
                if MODE[4 * j + h] == "P":
                    # DVE+Pool 2-op Schraudolph: w = max(S+bias,-88) on DVE
                    # (gpsimd cannot read PSUM); bits = trunc(A16*w + B16) on
                    # gpsimd -> int16 view of bf16 tile
                    tmp = big.tile([128, 512], f32, tag="stmp", name="stmp", bufs=3)
                    with tc.high_priority():
                        nc.vector.tensor_scalar(
                            out=tmp,
                            in0=S,
                            scalar1=st["bias"][:, j : j + 1],
                            scalar2=-88.0,
                            op0=ALU.add,
                            op1=ALU.max,
                        )
                    nc.gpsimd.tensor_scalar(
                        out=Eq[h].bitcast(i16),
                        in0=tmp,
                        scalar1=A16,
                        scalar2=B16,
                        op0=ALU.mult,
                        op1=ALU.add,
                    )
                else:
                    nc.scalar.activation(
                        out=Eq[h],
                        in_=S,
                        func=AF.Exp,
                        bias=st["bias"][:, j : j + 1],
                        scale=1.0,
                    )

        def emit_av(b, st, j):
            Eq = st["E"].pop(j)
            for jj in range(NT):
                lhsT = Eq[jj // 4][:, (jj % 4) * 128 : (jj % 4 + 1) * 128]
                # 4 output slices share a 2KB PSUM bank = one zero region:
                # only the bank's first MM sets start, only its last sets stop
                nc.tensor.matmul(
                    out=st["num"][:, jj * 128 : (jj + 1) * 128],
                    lhsT=lhsT,
                    rhs=st["Vt"][:, j, :],
                    start=(j == 0 and jj % 4 == 0),
                    stop=(j == NT - 1 and jj % 4 == 3),
                )
                # denominator: same stationary weights, t column as rhs ->
                # denP[q', jj] accumulates sum_k exp(S[q',k]-G) over all j
                nc.tensor.matmul(
                    out=st["denP"][:, jj : jj + 1],
                    lhsT=lhsT,
                    rhs=st["Tb"][:, j : j + 1],
                    start=(j == 0 and jj == 0),
                    stop=(j == NT - 1 and jj == NT - 1),
                )

        def emit_den(b, st):
            denS = stats.tile([128, NT], f32, tag="denS", name="denS")
            nc.vector.tensor_copy(out=denS, in_=st["denP"])
            R = stats.tile([128, NT], f32, tag="R", name="R")
            nc.vector.reciprocal(out=R, in_=denS)
            st["R"] = R

        def emit_drain(b, st, copy_psum=True, half=None):
            # drain AV results out of PSUM so the next batch can reuse it
            # (skipped for the last batch -- nothing needs the banks).
            # Staggered: half 0 at the phase boundary, half 1 a few iterations
            # later, so the copies don't starve DVE mid-phase.
            if copy_psum:
                if half != 1:
                    st["numS"] = big.tile([128, T], f32, tag="numS", name="numS")
                for h in ([0, 1] if half is None else [half]):
                    if h == 0:
                        nc.vector.tensor_copy(
                            out=st["numS"][:, 0:1024],
                            in_=st["num"][:, 0:1024],
                        )
                    else:
                        nc.scalar.activation(
                            out=st["numS"][:, 1024:2048],
                            in_=st["num"][:, 1024:2048],
                            func=AF.Copy,
                        )
            else:
                st["numS"] = st["num"]
            if half != 1:
                st["Y"] = big.tile([128, NT, D], f32, tag="Y", name="Y")
                st["Yout"] = big.tile([128, NT, D], f32, tag="Yout", name="Yout")
                st["Sy"] = stats.tile([128, NT], f32, tag="Sy", name="Sy")
                st["Sy2"] = stats.tile([128, NT], f32, tag="Sy2", name="Sy2")
                st["ysqb"] = big.tile([128, NT, D], f32, tag="ysqb", name="ysqb")

        def emit_outA(b, st, jj, tail=False):
            # y = num'/den' + x, with LN stats via accum_out.  The y-compute
            # reads numS (PSUM for the last batch) so it goes on DVE unless
            # numS was drained to SBUF, in which case odd tiles go on gpsimd.
            nc.vector.scalar_tensor_tensor(
                out=st["Y"][:, jj, :],
                in0=st["numS"][:, jj * 128 : (jj + 1) * 128],
                scalar=st["R"][:, jj : jj + 1],
                in1=st["x"][:, jj, :],
                op0=ALU.mult,
                op1=ALU.add,
                accum_out=st["Sy"][:, jj : jj + 1],
            )
            if tail and jj % 2 == 0:
                nc.scalar.activation(
                    out=st["ysqb"][:, jj, :],
                    in_=st["Y"][:, jj, :],
                    func=AF.Square,
                    accum_out=st["Sy2"][:, jj : jj + 1],
                )
            else:
                nc.vector.scalar_tensor_tensor(
                    out=st["ysqb"][:, jj, :],
                    in0=st["Y"][:, jj, :],
                    scalar=1.0,
                    in1=st["Y"][:, jj, :],
                    op0=ALU.mult,
                    op1=ALU.mult,
                    accum_out=st["Sy2"][:, jj : jj + 1],
                )

        def emit_lnr(b, st, lo=0, hi=NT):
            cs = slice(lo, hi)
            if True:
                if "mu" not in st:
                    st["mu"] = stats.tile([128, NT], f32, tag="mu", name="mu")
                    st["vart"] = stats.tile([128, NT], f32, tag="vart", name="vart")
                    st["rstd"] = stats.tile([128, NT], f32, tag="rstd", name="rstd")
                # mu = Sy/128, var = Sy2/128 - mu^2
                nc.vector.tensor_scalar_mul(
                    out=st["mu"][:, cs], in0=st["Sy"][:, cs], scalar1=1.0 / D
                )
                musq = stats.tile([128, NT], f32, tag="musq", name="musq")
                nc.vector.scalar_tensor_tensor(
                    out=musq[:, cs],
                    in0=st["mu"][:, cs],
                    scalar=1.0,
                    in1=st["mu"][:, cs],
                    op0=ALU.mult,
                    op1=ALU.mult,
                )
                nc.vector.scalar_tensor_tensor(
                    out=st["vart"][:, cs],
                    in0=st["Sy2"][:, cs],
                    scalar=1.0 / D,
                    in1=musq[:, cs],
                    op0=ALU.mult,
                    op1=ALU.subtract,
                )
                var_in = st["vart"][:, cs]
            # rstd = 1/sqrt(var+eps) via the fast-inverse-sqrt bit trick plus
            # two Newton steps (~4e-6 rel err) -- keeps the ACT engine on the
            # exp table set for the whole kernel (table swaps cost 1.3us each)
            ve = stats.tile([128, NT], f32, tag="ve", name="ve")
            nc.vector.tensor_scalar_add(out=ve[:, cs], in0=var_in, scalar1=EPS)
            wf = stats.tile([128, NT], f32, tag="wf", name="wf")
            nc.vector.tensor_copy(out=wf[:, cs], in_=ve[:, cs].bitcast(mybir.dt.int32))
            nc.vector.tensor_scalar(
                out=wf[:, cs], in0=wf[:, cs],
                scalar1=-0.5, scalar2=1597463007.0,
                op0=ALU.mult, op1=ALU.add,
            )
            wi = stats.tile([128, NT], mybir.dt.int32, tag="wi", name="wi")
            nc.vector.tensor_copy(out=wi[:, cs], in_=wf[:, cs])
            y = stats.tile([128, NT], f32, tag="y0", name="y0")
            nc.vector.tensor_copy(out=y[:, cs], in_=wi[:, cs].bitcast(f32))
            t1 = stats.tile([128, NT], f32, tag="t1", name="t1")
            for it in range(2):
                nc.vector.tensor_mul(out=t1[:, cs], in0=ve[:, cs], in1=y[:, cs])
                nc.vector.tensor_mul(out=t1[:, cs], in0=t1[:, cs], in1=y[:, cs])
                nc.vector.tensor_scalar(
                    out=t1[:, cs], in0=t1[:, cs],
                    scalar1=-0.5, scalar2=1.5, op0=ALU.mult, op1=ALU.add,
                )
                nc.vector.tensor_mul(
                    out=st["rstd"][:, cs] if it == 1 else y[:, cs],
                    in0=y[:, cs], in1=t1[:, cs],
                )

        def emit_outT(b, st, jj):
            # tail path: ACT scales num by R (idle engine there), Pool adds the
            # residual, DVE computes LN stats via bn_stats/bn_aggr
            nc.scalar.activation(
                out=st["Y1"][:, jj, :],
                in_=st["num"][:, jj * 128 : (jj + 1) * 128],
                func=AF.Copy,
                scale=st["R"][:, jj : jj + 1],
            )
            nc.gpsimd.tensor_add(
                out=st["Y"][:, jj, :], in0=st["Y1"][:, jj, :], in1=st["x"][:, jj, :]
            )
            bns = stats.tile([128, 6], f32, tag="bns", name="bns")
            nc.vector.bn_stats(out=bns, in_=st["Y"][:, jj, :])
            nc.vector.bn_aggr(out=st["MV"][:, jj, :], in_=bns)

        def emit_lnr_mv(b, st, lo=0, hi=NT):
            cs = slice(lo, hi)
            if "rstd" not in st:
                st["rstd"] = stats.tile([128, NT], f32, tag="rstd", name="rstd")
                st["ve"] = stats.tile([128, NT], f32, tag="ve", name="ve")
                st["wf"] = stats.tile([128, NT], f32, tag="wf", name="wf")
                st["wi"] = stats.tile([128, NT], mybir.dt.int32, tag="wi", name="wi")
                st["y0"] = stats.tile([128, NT], f32, tag="y0", name="y0")
                st["t1"] = stats.tile([128, NT], f32, tag="t1", name="t1")
            ve = st["ve"]
            nc.vector.tensor_scalar_add(out=ve[:, cs], in0=st["MV"][:, cs, 1], scalar1=EPS)
            wf = st["wf"]
            nc.vector.tensor_copy(out=wf[:, cs], in_=ve[:, cs].bitcast(mybir.dt.int32))
            nc.vector.tensor_scalar(
                out=wf[:, cs], in0=wf[:, cs],
                scalar1=-0.5, scalar2=1597463007.0,
                op0=ALU.mult, op1=ALU.add,
            )
            wi = st["wi"]
            nc.vector.tensor_copy(out=wi[:, cs], in_=wf[:, cs])
            y = st["y0"]
            nc.vector.tensor_copy(out=y[:, cs], in_=wi[:, cs].bitcast(f32))
            t1 = st["t1"]
            for it in range(2):
                nc.vector.tensor_mul(out=t1[:, cs], in0=ve[:, cs], in1=y[:, cs])
                nc.vector.tensor_mul(out=t1[:, cs], in0=t1[:, cs], in1=y[:, cs])
                nc.vector.tensor_scalar(
                    out=t1[:, cs], in0=t1[:, cs],
                    scalar1=-0.5, scalar2=1.5, op0=ALU.mult, op1=ALU.add,
                )
                nc.vector.tensor_mul(
                    out=st["rstd"][:, cs] if it == 1 else y[:, cs],
                    in0=y[:, cs], in1=t1[:, cs],
                )

        def emit_outB(b, st, jj, tail=False):
            mu_s = st["MV"][:, jj, 0:1] if "MV" in st else st["mu"][:, jj : jj + 1]
            z = stats.tile([128, D], f32, tag="z", name="z")
            zeng = nc.vector if (tail and jj >= 12 and jj % 2 == 0) else nc.gpsimd
            zeng.tensor_scalar(
                out=z,
                in0=st["Y"][:, jj, :],
                scalar1=mu_s,
                scalar2=st["rstd"][:, jj : jj + 1],
                op0=ALU.subtract,
                op1=ALU.mult,
            )
            z2 = stats.tile([128, D], f32, tag="z2", name="z2")
            zeng.tensor_mul(out=z2, in0=z, in1=gb)
            zeng.tensor_add(out=st["Yout"][:, jj, :], in0=z2, in1=bb)

        def emit_outdma(b, st, half=None, quarter=None, eng=None):
            ov = o_d[b].rearrange("(t p) d -> p t d", p=128)
            eng = eng or nc.sync
            if quarter is not None:
                q4 = slice(quarter * 4, (quarter + 1) * 4)
                eng.dma_start(out=ov[:, q4, :], in_=st["Yout"][:, q4, :])
            elif half is None:
                eng.dma_start(out=ov, in_=st["Yout"])
            else:
                h8 = slice(half * 8, (half + 1) * 8)
                eng.dma_start(out=ov[:, h8, :], in_=st["Yout"][:, h8, :])

        # ---- software-pipelined schedule over the two batches ---------------
        A, Bst = {}, {}
        emit_loads(0, A)
        emit_loads_xT(0, A)
        emit_stats_a(0, A)
        emit_stats_b(0, A)
        emit_loads(1, Bst)
        emit_loads_xT(1, Bst)
        gb = consts.tile([128, D], f32, tag="gb", name="gb")
        bb = consts.tile([128, D], f32, tag="bb", name="bb")
        for j in range(NT):
            if j in (4, 8, 12):
                emit_stats_q(0, A, j // 4)
            emit_qk_exp(0, A, j)
            if j > 0:
                emit_av(0, A, j - 1)
            if j == 8:
                with tc.tile_wait_until(0.018):
                    emit_stats_a(1, Bst)
                    emit_stats_b(1, Bst)
            if j in (10, 12, 14):
                with tc.tile_wait_until(0.018 + 0.002 * (j - 8)):
                    emit_stats_q(1, Bst, (j - 8) // 2)
            if j == 5:
                nc.sync.dma_start(out=gb, in_=g_d[:].partition_broadcast(128))
                nc.sync.dma_start(out=bb, in_=b_d[:].partition_broadcast(128))
        emit_av(0, A, NT - 1)
        emit_den(0, A)
        emit_drain(0, A)
        # phase 1: batch 1's main loop with batch 0's whole output stage
        # threaded through it (outA x2 in early iters, lnr at 8, outB x2 late)
        for j in range(NT):
            emit_qk_exp(1, Bst, j)
            if j > 0:
                emit_av(1, Bst, j - 1)
            if 3 <= j < 11:
                emit_outA(0, A, 2 * (j - 3))
                emit_outA(0, A, 2 * (j - 3) + 1)
            elif j >= 11:
                if j == 11:
                    emit_lnr(0, A)
                for q in range(3):
                    emit_outB(0, A, 3 * (j - 11) + q)
                if j == 14:
                    emit_outdma(0, A, half=0)
        emit_av(1, Bst, NT - 1)
        emit_outB(0, A, 15)
        emit_outdma(0, A, half=1)
        Bst["numS"] = Bst["num"]
        Bst["Y"] = big.tile([128, NT, D], f32, tag="Y", name="Y")
        Bst["Y1"] = big.tile([128, NT, D], f32, tag="Y1", name="Y1", bufs=1)
        Bst["Yout"] = big.tile([128, NT, D], f32, tag="Yout", name="Yout")
        Bst["MV"] = stats.tile([128, NT, 2], f32, tag="MV", name="MV")
        # quarter-pipelined tail: the ACT y1 stream leads; each quarter's
        # lnr/outB/store trails one quarter behind so no engine head-of-line
        # blocks another
        Bst["denS"] = stats.tile([128, NT], f32, tag="denS", name="denS")
        Bst["R"] = stats.tile([128, NT], f32, tag="R", name="R")
        for q in range(5):
            if q < 4:
                qs = slice(4 * q, 4 * q + 4)
                nc.vector.tensor_copy(out=Bst["denS"][:, qs], in_=Bst["denP"][:, qs])
                nc.vector.reciprocal(out=Bst["R"][:, qs], in_=Bst["denS"][:, qs])
            if q >= 1:
                with tc.high_priority():
                    emit_lnr_mv(1, Bst, lo=4 * (q - 1), hi=4 * q)
                    for jj in range(4 * (q - 1), 4 * q):
                        emit_outB(1, Bst, jj, tail=True)
                if q == 4:
                    ov = o_d[1].rearrange("(t p) d -> p t d", p=128)
                    nc.scalar.dma_start(
                        out=ov[:, 12:14, :], in_=Bst["Yout"][:, 12:14, :]
                    )
                    nc.sync.dma_start(
                        out=ov[:, 14:16, :], in_=Bst["Yout"][:, 14:16, :]
                    )
                else:
                    emit_outdma(1, Bst, quarter=q - 1, eng=nc.sync)
            if q < 4:
                for jj in range(4 * q, 4 * q + 4):
                    emit_outT(1, Bst, jj)

    nc.finalize()
    return nc


def _get_nc():
    if "nc" not in _CACHE:
        _CACHE["nc"] = _build()
    return _CACHE["nc"]


def _run(x, gamma, beta, trace=False):
    import ml_dtypes

    from concourse.bass_utils import run_bass_kernel_spmd

    x = np.ascontiguousarray(np.asarray(x, dtype=np.float32))
    gamma = np.ascontiguousarray(np.asarray(gamma, dtype=np.float32))
    beta = np.ascontiguousarray(np.asarray(beta, dtype=np.float32))

    xs = x.reshape(N_CORES, NB, T, D)
    xbf = xs.astype(ml_dtypes.bfloat16)
    xTs = np.ascontiguousarray(xs.transpose(0, 1, 3, 2)).astype(ml_dtypes.bfloat16)

    in_maps = [
        {
            "x": np.ascontiguousarray(xbf[c]),
            "xT": xTs[c],
            "gamma": gamma,
            "beta": beta,
        }
        for c in range(N_CORES)
    ]
    res = run_bass_kernel_spmd(
        _get_nc(), in_maps, core_ids=list(range(N_CORES)), trace=trace
    )
    out = np.stack([res.results[c]["out"] for c in range(N_CORES)], axis=0)
    return out.reshape(B, T, D), res


def kernel(x, gamma, beta):
    out, _ = _run(x, gamma, beta, trace=False)
    return out
